# revision 1
# baseline (speedup 1.0000x reference)
"""Trainium2 Bass kernel for ContinuousFilterConvolution (SchNet cfconv).

Reference computation (per edge e with source node s = src[e]):
    h    = ssp(rbf @ w1 + b1)            # [E, CH] filter MLP layer 1
    w    = ssp(h @ w2 + b2)              # [E, CH] filter MLP layer 2
    f    = x @ w3                        # [N, CH]
    wf   = w * f[src]                    # [E, CH]
    conv = segment_sum(wf, src)          # [N, CH]
    y    = ssp(conv @ w4 + b4)
    out  = x + (y @ w5 + b5)
with ssp(x) = softplus(x) - log(2).

Key algebraic simplification: gather and scatter use the SAME index, so
    conv[n] = (sum_{src[e]=n} w[e]) * f[n]
i.e. only the filter weights w need a segment-sum -- no per-edge gather of f.

Distribution: nodes are split into 8 equal shards; each core receives exactly
the edges whose source node lies in its shard (host-side bucketing).  No
collectives are needed: each core computes the full output rows of its node
shard and the host concatenates.

Per-core layout (all built on host):
  - Edges are bucketed into node-blocks of 128 consecutive local nodes.
    Every block gets a fixed slot capacity (shared across cores, multiple of
    128); slack slots are padding (rbf rows = 0, srcrel = -1e9 so the one-hot
    scatter matrix row is all zero).
  - The segment-sum becomes, per 128-edge subtile, a one-hot matmul
        scat[ch, n_blk] += w_subtile[e, ch].T @ O[e, n_blk]
    with O[e, j] = (srcrel[e] == j) built on the fly by one DVE tensor_scalar
    is_equal op against a constant iota tile; PSUM accumulates across the
    block's subtiles.
  - ssp(z) = Ln(0.5*Exp(z) + 0.5) exactly (the -log2 is absorbed by Ln's
    affine pre-scale).  Layer-2 bias is applied multiplicatively after Exp
    (e^(z+b) = e^z * e^b) to keep each quadrant matmul a self-contained PSUM
    accumulation group.

Self-contained: hardcodes the problem shapes; no file reads.
"""

import numpy as np
import ml_dtypes

import concourse.bacc as bacc
import concourse.mybir as mybir
import concourse.tile as tile
from concourse.bass_utils import run_bass_kernel_spmd

BF16_NP = ml_dtypes.bfloat16
F32 = mybir.dt.float32
BF16 = mybir.dt.bfloat16
AF = mybir.ActivationFunctionType
ALU = mybir.AluOpType

N_CORES = 8
CH = 128
RBF_DIM = 200
K1A = 128
K1B = RBF_DIM - K1A  # 72
SUB = 128            # edges per scatter subtile
MEGA = 1024          # edges per compute megatile
BLK = 128            # nodes per scatter block
LOG2 = 0.6931471805599453


# --------------------------------------------------------------------------
# host-side geometry
# --------------------------------------------------------------------------

def _geometry(src, n_nodes, n_cores):
    """Shared (all-core) slot geometry from the edge source indices."""
    nsh = n_nodes // n_cores
    assert nsh * n_cores == n_nodes
    nblk = (nsh + BLK - 1) // BLK
    npad = nblk * BLK
    core = src // nsh
    local = src - core * nsh
    blk = local // BLK
    flat = core * nblk + blk
    counts = np.bincount(flat, minlength=n_cores * nblk).reshape(n_cores, nblk)
    cap = np.maximum(counts.max(axis=0), 1)
    cap = ((cap + SUB - 1) // SUB) * SUB
    total = int(cap.sum())
    pad_total = (-total) % MEGA
    cap = cap.copy()
    cap[-1] += pad_total
    total += pad_total
    t_mega = total // MEGA
    offs = np.concatenate([[0], np.cumsum(cap)]).astype(np.int64)
    n_sub = total // SUB
    sub_starts = np.arange(n_sub, dtype=np.int64) * SUB
    sub_block = np.searchsorted(offs, sub_starts, side="right") - 1
    sub_first = sub_starts == offs[sub_block]
    sub_last = (sub_starts + SUB) == offs[sub_block + 1]
    return dict(
        nsh=nsh, nblk=nblk, npad=npad, core=core, local=local, blk=blk,
        counts=counts, offs=offs, total=total, t_mega=t_mega,
        sub_block=sub_block.tolist(), sub_first=sub_first.tolist(),
        sub_last=sub_last.tolist(),
    )


def _per_core_inputs(c, g, rbf, x, weights):
    """Build the input map for one core."""
    nsh, npad, total, t_mega = g["nsh"], g["npad"], g["total"], g["t_mega"]
    offs, counts = g["offs"], g["counts"]
    core, local, blk = g["core"], g["local"], g["blk"]

    sel = np.flatnonzero(core == c)
    blk_c = blk[sel]
    order = np.argsort(blk_c, kind="stable")
    sel_o = sel[order]
    blk_o = blk_c[order]
    cnt = counts[c]
    start_in_sorted = np.concatenate([[0], np.cumsum(cnt)[:-1]])
    rank = np.arange(sel.size) - start_in_sorted[blk_o]
    slot = offs[blk_o] + rank

    rbf_slots = np.zeros((total, RBF_DIM), BF16_NP)
    rbf_slots[slot] = rbf[sel_o].astype(BF16_NP)
    srcrel_slots = np.full(total, -1.0e9, np.float32)
    srcrel_slots[slot] = (local[sel_o] - blk_o * BLK).astype(np.float32)

    rT = np.ascontiguousarray(
        rbf_slots.reshape(t_mega, MEGA, RBF_DIM).transpose(0, 2, 1))
    srcrel_t = np.ascontiguousarray(
        srcrel_slots.reshape(t_mega, MEGA // SUB, SUB).transpose(2, 0, 1)
    ).reshape(SUB, -1)

    xs = x[c * nsh:(c + 1) * nsh]  # [nsh, CH] f32
    xT = np.zeros((CH, npad), BF16_NP)
    xT[:, :nsh] = xs.T.astype(BF16_NP)
    xrows = np.zeros((npad, CH), np.float32)
    xrows[:nsh] = xs

    m = {
        "rbfA": np.ascontiguousarray(rT[:, :K1A, :]),
        "rbfB": np.ascontiguousarray(rT[:, K1A:, :]),
        "srcrel": srcrel_t,
        "xT": xT,
        "xrows": xrows,
    }
    m.update(weights)
    return m


def _shared_weight_inputs(w1, b1, w2, b2, w3, w4, b4, w5, b5):
    eb2 = np.exp(b2.astype(np.float64)).astype(np.float32)  # [CH]
    return {
        "w1a": w1[:K1A].astype(BF16_NP),
        "w1b": w1[K1A:].astype(BF16_NP),
        "w2c": w2.astype(BF16_NP),
        "w3c": w3.astype(BF16_NP),
        "w4c": w4.astype(BF16_NP),
        "w5c": w5.astype(BF16_NP),
        "b1c": b1.reshape(CH, 1).astype(np.float32),
        "b4c": b4.reshape(CH, 1).astype(np.float32),
        "b5r": b5.reshape(1, CH).astype(BF16_NP),
        "eb2w": np.ascontiguousarray(
            np.broadcast_to(np.tile(eb2, MEGA // CH), (SUB, MEGA))).astype(np.float32),
        "iotat": np.ascontiguousarray(
            np.broadcast_to(np.arange(SUB, dtype=np.float32), (SUB, SUB))
        ).astype(BF16_NP),
        "onesr": np.ones((1, CH), BF16_NP),
    }


# --------------------------------------------------------------------------
# program builder
# --------------------------------------------------------------------------

def _build_program(t_mega, npad, sub_block, sub_first, sub_last):
    nblk = npad // BLK
    n_sub = t_mega * (MEGA // SUB)
    nc = bacc.Bacc("TRN2", target_bir_lowering=False, debug=False)

    rbfA_d = nc.dram_tensor("rbfA", [t_mega, K1A, MEGA], BF16, kind="ExternalInput")
    rbfB_d = nc.dram_tensor("rbfB", [t_mega, K1B, MEGA], BF16, kind="ExternalInput")
    srcrel_d = nc.dram_tensor("srcrel", [SUB, n_sub], F32, kind="ExternalInput")
    xT_d = nc.dram_tensor("xT", [CH, npad], BF16, kind="ExternalInput")
    xrows_d = nc.dram_tensor("xrows", [npad, CH], F32, kind="ExternalInput")
    w1a_d = nc.dram_tensor("w1a", [K1A, CH], BF16, kind="ExternalInput")
    w1b_d = nc.dram_tensor("w1b", [K1B, CH], BF16, kind="ExternalInput")
    w2_d = nc.dram_tensor("w2c", [CH, CH], BF16, kind="ExternalInput")
    w3_d = nc.dram_tensor("w3c", [CH, CH], BF16, kind="ExternalInput")
    w4_d = nc.dram_tensor("w4c", [CH, CH], BF16, kind="ExternalInput")
    w5_d = nc.dram_tensor("w5c", [CH, CH], BF16, kind="ExternalInput")
    b1_d = nc.dram_tensor("b1c", [CH, 1], F32, kind="ExternalInput")
    b4_d = nc.dram_tensor("b4c", [CH, 1], F32, kind="ExternalInput")
    b5_d = nc.dram_tensor("b5r", [1, CH], BF16, kind="ExternalInput")
    eb2_d = nc.dram_tensor("eb2w", [SUB, MEGA], F32, kind="ExternalInput")
    iota_d = nc.dram_tensor("iotat", [SUB, SUB], BF16, kind="ExternalInput")
    ones_d = nc.dram_tensor("onesr", [1, CH], BF16, kind="ExternalInput")
    out_d = nc.dram_tensor("out", [npad, CH], F32, kind="ExternalOutput")

    with tile.TileContext(nc) as tc:
        with tc.tile_pool(name="const", bufs=1) as const:
            w1a = const.tile([K1A, CH], BF16)
            nc.sync.dma_start(w1a[:], w1a_d[:, :])
            w1b = const.tile([K1B, CH], BF16)
            nc.sync.dma_start(w1b[:], w1b_d[:, :])
            w2 = const.tile([CH, CH], BF16)
            nc.sync.dma_start(w2[:], w2_d[:, :])
            w3 = const.tile([CH, CH], BF16)
            nc.sync.dma_start(w3[:], w3_d[:, :])
            w4 = const.tile([CH, CH], BF16)
            nc.sync.dma_start(w4[:], w4_d[:, :])
            w5 = const.tile([CH, CH], BF16)
            nc.sync.dma_start(w5[:], w5_d[:, :])
            b1c = const.tile([CH, 1], F32)
            nc.sync.dma_start(b1c[:], b1_d[:, :])
            b4c = const.tile([CH, 1], F32)
            nc.sync.dma_start(b4c[:], b4_d[:, :])
            b5r = const.tile([1, CH], BF16)
            nc.sync.dma_start(b5r[:], b5_d[:, :])
            onesr = const.tile([1, CH], BF16)
            nc.sync.dma_start(onesr[:], ones_d[:, :])
            eb2w = const.tile([SUB, MEGA], F32)
            nc.sync.dma_start(eb2w[:], eb2_d[:, :])
            iotat = const.tile([SUB, SUB], BF16)
            nc.sync.dma_start(iotat[:], iota_d[:, :])
            srcrel = const.tile([SUB, n_sub], F32)
            nc.sync.dma_start(srcrel[:], srcrel_d[:, :])
            xT = const.tile([CH, npad], BF16)
            nc.sync.dma_start(xT[:], xT_d[:, :])
            half = const.tile([CH, 1], F32)
            nc.any.memset(half[:], 0.5)

            fT = const.tile([CH, npad], F32)
            convT = const.tile([CH, npad], BF16)
            yT = const.tile([CH, npad], BF16)

            # ---- stage 1: fT = (x @ w3)^T -------------------------------
            with tc.tile_pool(name="fpsum", bufs=2, space="PSUM") as fpsum:
                for c0 in range(0, npad, 512):
                    w = min(512, npad - c0)
                    fp = fpsum.tile([CH, 512], F32, tag="fp")
                    nc.tensor.matmul(fp[:, :w], w3[:], xT[:, c0:c0 + w],
                                     start=True, stop=True)
                    nc.vector.tensor_copy(fT[:, c0:c0 + w], fp[:, :w])

            # ---- stage 2: edge loop -------------------------------------
            with (
                tc.tile_pool(name="rbfa_p", bufs=3) as rbfa_p,
                tc.tile_pool(name="rbfb_p", bufs=3) as rbfb_p,
                tc.tile_pool(name="e1_p", bufs=2) as e1_p,
                tc.tile_pool(name="e2_p", bufs=2) as e2_p,
                tc.tile_pool(name="h_p", bufs=2) as h_p,
                tc.tile_pool(name="w_p", bufs=2) as w_p,
                tc.tile_pool(name="o_p", bufs=4) as o_p,
                tc.tile_pool(name="z1_p", bufs=2, space="PSUM") as z1_p,
                tc.tile_pool(name="z2_p", bufs=1, space="PSUM") as z2_p,
                tc.tile_pool(name="scat_p", bufs=2, space="PSUM") as scat_p,
            ):
                scat_cur = None
                for m in range(t_mega):
                    ra = rbfa_p.tile([K1A, MEGA], BF16, tag="ra")
                    nc.sync.dma_start(ra[:], rbfA_d[m, :, :])
                    rb = rbfb_p.tile([K1B, MEGA], BF16, tag="rb")
                    nc.sync.dma_start(rb[:], rbfB_d[m, :, :])

                    z1 = z1_p.tile([CH, MEGA], F32, tag="z1")
                    for h0 in range(0, MEGA, 512):
                        sl = slice(h0, h0 + 512)
                        nc.tensor.matmul(z1[:, sl], w1a[:], ra[:, sl],
                                         start=True, stop=False)
                        nc.tensor.matmul(z1[:, sl], w1b[:], rb[:, sl],
                                         start=False, stop=True)
                    e1 = e1_p.tile([CH, MEGA], F32, tag="e1")
                    nc.scalar.activation(e1[:], z1[:], AF.Exp, bias=b1c[:])
                    hT = h_p.tile([CH, MEGA], BF16, tag="h")
                    nc.scalar.activation(hT[:], e1[:], AF.Ln,
                                         bias=half[:], scale=0.5)

                    z2 = z2_p.tile([SUB, MEGA], F32, tag="z2")
                    for j in range(MEGA // SUB):
                        sl = slice(SUB * j, SUB * j + SUB)
                        nc.tensor.matmul(z2[:, sl], hT[:, sl], w2[:],
                                         start=True, stop=True)
                    e2 = e2_p.tile([SUB, MEGA], F32, tag="e2")
                    nc.scalar.activation(e2[:], z2[:], AF.Exp)
                    nc.vector.tensor_tensor(e2[:], e2[:], eb2w[:], ALU.mult)
                    wsp = w_p.tile([SUB, MEGA], BF16, tag="wsp")
                    nc.scalar.activation(wsp[:], e2[:], AF.Ln,
                                         bias=half[:], scale=0.5)

                    for j in range(MEGA // SUB):
                        gs = (MEGA // SUB) * m + j
                        b = sub_block[gs]
                        if sub_first[gs]:
                            scat_cur = scat_p.tile([CH, BLK], F32, tag="scat")
                        O = o_p.tile([SUB, BLK], BF16, tag="O")
                        nc.vector.tensor_scalar(
                            O[:], iotat[:], srcrel[:, gs:gs + 1], None,
                            ALU.is_equal)
                        nc.tensor.matmul(scat_cur[:],
                                         wsp[:, SUB * j:SUB * j + SUB], O[:],
                                         start=bool(sub_first[gs]),
                                         stop=bool(sub_last[gs]))
                        if sub_last[gs]:
                            nc.vector.tensor_tensor(
                                convT[:, BLK * b:BLK * b + BLK], scat_cur[:],
                                fT[:, BLK * b:BLK * b + BLK], ALU.mult)

            # ---- stage 3: node MLP + residual ---------------------------
            with (
                tc.tile_pool(name="e4_p", bufs=2) as e4_p,
                tc.tile_pool(name="xr_p", bufs=3) as xr_p,
                tc.tile_pool(name="or_p", bufs=3) as or_p,
                tc.tile_pool(name="z4_p", bufs=2, space="PSUM") as z4_p,
                tc.tile_pool(name="v_p", bufs=3, space="PSUM") as v_p,
            ):
                for c0 in range(0, npad, 512):
                    w = min(512, npad - c0)
                    z4 = z4_p.tile([CH, 512], F32, tag="z4")
                    nc.tensor.matmul(z4[:, :w], w4[:], convT[:, c0:c0 + w],
                                     start=True, stop=True)
                    e4 = e4_p.tile([CH, 512], F32, tag="e4")
                    nc.scalar.activation(e4[:, :w], z4[:, :w], AF.Exp,
                                         bias=b4c[:])
                    nc.scalar.activation(yT[:, c0:c0 + w], e4[:, :w], AF.Ln,
                                         bias=half[:], scale=0.5)
                for jb in range(nblk):
                    sl = slice(BLK * jb, BLK * jb + BLK)
                    v = v_p.tile([BLK, CH], F32, tag="v")
                    nc.tensor.matmul(v[:], yT[:, sl], w5[:],
                                     start=True, stop=False)
                    nc.tensor.matmul(v[:], onesr[:], b5r[:],
                                     start=False, stop=True)
                    xr = xr_p.tile([BLK, CH], F32, tag="xr")
                    nc.sync.dma_start(xr[:], xrows_d[sl, :])
                    orow = or_p.tile([BLK, CH], F32, tag="orow")
                    nc.vector.tensor_tensor(orow[:], v[:], xr[:], ALU.add)
                    nc.sync.dma_start(out_d[sl, :], orow[:])

    nc.compile()
    return nc


# --------------------------------------------------------------------------
# public entry point
# --------------------------------------------------------------------------

def _prepare(rbf, x, src, w1, b1, w2, b2, w3, w4, b4, w5, b5,
             n_cores=N_CORES):
    rbf = np.asarray(rbf, np.float32)
    x = np.asarray(x, np.float32)
    src = np.asarray(src).astype(np.int64)
    g = _geometry(src, x.shape[0], n_cores)
    weights = _shared_weight_inputs(
        np.asarray(w1, np.float32), np.asarray(b1, np.float32),
        np.asarray(w2, np.float32), np.asarray(b2, np.float32),
        np.asarray(w3, np.float32), np.asarray(w4, np.float32),
        np.asarray(b4, np.float32), np.asarray(w5, np.float32),
        np.asarray(b5, np.float32))
    in_maps = [_per_core_inputs(c, g, rbf, x, weights)
               for c in range(n_cores)]
    return g, in_maps


def kernel(**inputs):
    g, in_maps = _prepare(**inputs)
    nc = _build_program(g["t_mega"], g["npad"], g["sub_block"],
                        g["sub_first"], g["sub_last"])
    res = run_bass_kernel_spmd(nc, in_maps, core_ids=list(range(N_CORES)))
    nsh = g["nsh"]
    out = np.concatenate(
        [res.results[c]["out"][:nsh] for c in range(N_CORES)], axis=0)
    return np.ascontiguousarray(out, dtype=np.float32)
